# revision 44
# baseline (speedup 1.0000x reference)
"""Trainium2 Bass kernel for multi-head attention (B=8, N=1024, DM=512, H=8, D=64).

Sharding: data-parallel over batch — core i handles batch element i, weights
replicated, no collectives.

All matmul operands are float32r (tf32-like, 1 cyc/row, self-loading
weights).  16-bit matmul operands were measured ~170ns/matmul SLOWER on
real hardware (walrus emits a separate Ldweights per 16-bit matmul), so
fp16 is used only for the output store; inputs stay f32.

Per-core pipeline:
  - qT, kT = W^T @ x^T  (weights stationary)          [512 rows, 1024 tok]
  - v natural [1024, 512] (hiddenT chunks stationary), stored ones-augmented
    as [128, 8, 65] per token-chunk (col 64 = 1.0 -> softmax sums for free;
    the column is memset on GpSimd, no DMA)
  - per (q-chunk, head-pair): dotsT [keys, queries], two heads packed in the
    two banks of a [128,2,512] psum tile; exp on ScalarE (no max subtraction
    -- logits are O(+-60), f32 exp cannot overflow) -> SBUF f32r
  - PV: out_h^T[65, 512] = v_aug^T @ expT, row 64 = softmax denominators
  - normalize: DVE reciprocal reads the psum sums row (lane 64); both heads'
    recip rows go to one [65,2,512] tile, ONE dma hops that row to a
    partition-0 tile (partition_broadcast's HW microcode reads partition 0
    regardless of the input AP's partition offset -- verified by micro-test),
    ONE [64,1024] partition_broadcast serves both heads, DVE multiplies.
    The PV psums are copied to SBUF in parallel so the next job's PV is not
    gated by the normalize chain.  Head-odd results are computed in lanes
    0:64 then DMA'd into oT partitions 64:128 (DVE lanes cannot cross
    partitions).
  - out-proj: out[t-chunk, :] = sum_ic oT[ic, t-chunk]^T @ Wo[ic, :],
    psum f32, stored fp16 (host converts back to f32).  qc=0 tiles run
    fully pre-norm at the drain; qc=1 tiles pre-start ic 0..2 partials in a
    6-buffer psum pool freed by the shared proj/dots pool.
  - startup: eager qk(0) is emitted interleaved per k-round in DMA-arrival
    order (the PE executes strictly in order, so emission order must match
    input arrival to avoid head-of-line blocking); qk(1) follows eagerly;
    vproj fills job 0's slots (Wkv-V arrives mid-job-0), qk(2)/(3) fill
    job 1.  Loads are split across the SP/ACT HWDGE queues and the Pool
    SWDGE queue so descriptor generation keeps ahead of the shared pipe.
"""
import sys

sys.path.insert(0, "/opt/trn_rl_repo")

import numpy as np

import concourse.bass as bass  # noqa: F401  (import keeps bass registered)
import concourse.mybir as mybir
import concourse.tile as tile
from concourse import bacc
from concourse.bass_utils import run_bass_kernel_spmd

F32 = mybir.dt.float32
F32R = mybir.dt.float32r
F16 = mybir.dt.float16
BF16 = mybir.dt.bfloat16

B, N, DM = 8, 1024, 512
H, D = 8, 64
NCORES = 8

_nc_cache = {}


def build_nc(loop_n=None):
    key = ("nc", loop_n)
    if key in _nc_cache:
        return _nc_cache[key]
    nc = bacc.Bacc("TRN2", target_bir_lowering=False, debug=False)

    curT_d = nc.dram_tensor("currentT", [DM, N], F32R, kind="ExternalInput").ap()
    hidT_d = nc.dram_tensor("hiddenT", [DM, N], F32R, kind="ExternalInput").ap()
    wq_d = nc.dram_tensor("Wq", [DM, H * D], F32R, kind="ExternalInput").ap()
    wkv_d = nc.dram_tensor("Wkv", [DM, 2 * H * D], F32R, kind="ExternalInput").ap()
    wo_d = nc.dram_tensor("Wo", [H * D, H * D], F32R, kind="ExternalInput").ap()
    out_d = nc.dram_tensor("out", [N, H * D], F16, kind="ExternalOutput").ap()

    with tile.TileContext(nc) as tc:
        if loop_n is None:
            build_body(nc, tc, curT_d, hidT_d, wq_d, wkv_d, wo_d, out_d)
        else:
            with tc.For_i(0, loop_n, 1):
                build_body(nc, tc, curT_d, hidT_d, wq_d, wkv_d, wo_d, out_d)
    nc.compile()
    _nc_cache[key] = nc
    return nc


def build_body(nc, tc, curT_d, hidT_d, wq_d, wkv_d, wo_d, out_d):
    import contextlib

    ctx = contextlib.ExitStack()
    with ctx:
        # ---------- pools ----------
        wpool = ctx.enter_context(tc.tile_pool(name="weights", bufs=1))
        actpool = ctx.enter_context(tc.tile_pool(name="acts", bufs=1))
        qkpool = ctx.enter_context(tc.tile_pool(name="qk", bufs=1))
        vpool = ctx.enter_context(tc.tile_pool(name="vaug", bufs=1))
        opool = ctx.enter_context(tc.tile_pool(name="ot", bufs=1))
        epool = ctx.enter_context(tc.tile_pool(name="expT", bufs=10))
        stgpool = ctx.enter_context(tc.tile_pool(name="stg", bufs=2))
        pvsb = ctx.enter_context(tc.tile_pool(name="pvsb", bufs=3))
        recpool = ctx.enter_context(tc.tile_pool(name="rec", bufs=2))
        bcpool = ctx.enter_context(tc.tile_pool(name="bcast", bufs=2))
        outsb = ctx.enter_context(tc.tile_pool(name="outsb", bufs=4))
        pvps = ctx.enter_context(tc.tile_pool(name="pvps", bufs=2, space="PSUM"))
        # one shared triple-buffered psum pool serves both the projection
        # accumulations and the dots tiles (same [128,2,512] shape)
        mmps_cm = tc.tile_pool(name="mmps", bufs=3, space="PSUM")
        mmps = mmps_cm.__enter__()
        state = {}

        # ---------- input loads ----------
        # SP queue: Wq + currentT interleaved, then Wkv-V half.
        # DVE queue: Wkv-K half + hiddenT interleaved, then Wo.
        # ScalarE queue stays clean for activations.
        wq = [wpool.tile([128, 512], F32R, tag=f"wq{k}", name=f"wq{k}") for k in range(4)]
        wkv = [wpool.tile([128, 1024], F32R, tag=f"wkv{k}", name=f"wkv{k}") for k in range(4)]
        wo = [wpool.tile([128, 512], F32R, tag=f"wo{k}", name=f"wo{k}") for k in range(4)]
        curT = [actpool.tile([128, 1024], F32R, tag=f"cur{k}", name=f"cur{k}") for k in range(4)]
        hidT = [actpool.tile([128, 1024], F32R, tag=f"hid{k}", name=f"hid{k}") for k in range(4)]

        # Descriptor generation must keep ahead of the shared DMA pipe:
        # spread the critical 4MB across all three dispatch paths (SP+ACT
        # HWDGE, Pool SWDGE).  ACT only carries loads that finish before the
        # first exp dispatches; SP carries the trailing non-critical loads.
        # Wq/Wkv-K load in column halves so the m=0/1 slices that unblock
        # qproj(0)/kproj(0) and the first dots arrive ~2.5us earlier.
        # Arrival order matches the eager emission order below: per-k rounds
        # of (Wq-mhalf0, cur, Wkv-mhalf0, hid) feed qk(0)/qk(1) immediately;
        # Wkv-V next (job-0 vproj fillers), m-half1 after (job-1 qk(2)/(3)
        # fillers), Wo last (only needed at the drain).
        for k in range(4):
            nc.sync.dma_start(wq[k][:, 0:256], wq_d[k * 128:(k + 1) * 128, 0:256])
            for n2 in range(2):
                nc.scalar.dma_start(
                    curT[k][:, n2 * 512:(n2 + 1) * 512],
                    curT_d[k * 128:(k + 1) * 128, n2 * 512:(n2 + 1) * 512])
            nc.sync.dma_start(wkv[k][:, 0:256],
                              wkv_d[k * 128:(k + 1) * 128, 0:256])
            for n2 in range(2):
                nc.gpsimd.dma_start(
                    hidT[k][:, n2 * 512:(n2 + 1) * 512],
                    hidT_d[k * 128:(k + 1) * 128, n2 * 512:(n2 + 1) * 512])
        for k in range(4):
            (nc.sync if k % 2 else nc.gpsimd).dma_start(
                wkv[k][:, 512:1024], wkv_d[k * 128:(k + 1) * 128, 512:1024])
        for k in range(4):
            nc.sync.dma_start(wq[k][:, 256:512],
                              wq_d[k * 128:(k + 1) * 128, 256:512])
            nc.gpsimd.dma_start(wkv[k][:, 256:512],
                                wkv_d[k * 128:(k + 1) * 128, 256:512])
        for k in range(4):
            nc.sync.dma_start(wo[k][:], wo_d[k * 128:(k + 1) * 128, :])

        # ---------- persistent tensors ----------
        qT = [qkpool.tile([128, 1024], F32R, tag=f"qT{m}", name=f"qT{m}") for m in range(4)]
        kT = [qkpool.tile([128, 1024], F32R, tag=f"kT{m}", name=f"kT{m}") for m in range(4)]
        vaug = [vpool.tile([128, H, D + 1], F32R, tag=f"va{t}", name=f"va{t}") for t in range(8)]
        oT = [opool.tile([128, 1024], F32R, tag=f"oT{i}", name=f"oT{i}") for i in range(4)]

        # ---------- projection units (emitted eagerly or as fillers) ----------
        # Each projection unit is split into two half-emissions (4 matmuls
        # each) so a filler only delays the next dots tile by ~0.85us and
        # the exp stream on ScalarE stays fed.
        def emit_qproj(m, half=None):
            if half in (0, None):
                state["psq"] = mmps.tile([128, 2, 512], F32, tag="mm",
                                         name="psq")
            ps = state["psq"]
            ks = range(4) if half is None else range(2 * half, 2 * half + 2)
            for k in ks:
                for n2 in range(2):
                    nc.tensor.matmul(
                        ps[:, n2, :],
                        wq[k][:, m * 128:(m + 1) * 128],
                        curT[k][:, n2 * 512:(n2 + 1) * 512],
                        start=(k == 0), stop=(k == 3))
            if half in (1, None):
                nc.vector.tensor_copy(
                    qT[m][:].rearrange("p (a b) -> p a b", a=2), ps[:])

        def emit_kproj(m, half=None):
            if half in (0, None):
                state["psk"] = mmps.tile([128, 2, 512], F32, tag="mm",
                                         name="psk")
            ps = state["psk"]
            ks = range(4) if half is None else range(2 * half, 2 * half + 2)
            for k in ks:
                for n2 in range(2):
                    nc.tensor.matmul(
                        ps[:, n2, :],
                        wkv[k][:, m * 128:(m + 1) * 128],
                        hidT[k][:, n2 * 512:(n2 + 1) * 512],
                        start=(k == 0), stop=(k == 3))
            if half in (1, None):
                nc.vector.tensor_copy(
                    kT[m][:].rearrange("p (a b) -> p a b", a=2), ps[:])

        def emit_vproj(tp, half=None):
            if half in (0, None):
                state["psv"] = mmps.tile([128, 2, 512], F32, tag="mm",
                                         name="psv")
            ps = state["psv"]
            t2s = range(2) if half is None else (half,)
            for t2 in t2s:
                tc_i = tp * 2 + t2
                for k in range(4):
                    nc.tensor.matmul(
                        ps[:, t2, :],
                        hidT[k][:, tc_i * 128:(tc_i + 1) * 128],
                        wkv[k][:, 512:1024],
                        start=(k == 0), stop=(k == 3))
                nc.vector.tensor_copy(
                    vaug[tc_i][:, :, 0:D],
                    ps[:, t2, :].rearrange("p (h d) -> p h d", h=H))
                nc.gpsimd.memset(vaug[tc_i][:, :, D:D + 1].bitcast(F32), 1.0)

        # ---------- attention helpers ----------
        def emit_pv(js, kc):
            if kc == 0:
                js["pve"] = pvps.tile([D + 1, 512], F32, tag="pv", name="pve")
                js["pvo"] = pvps.tile([D + 1, 512], F32, tag="pv", name="pvo")
            et = js["etiles"][kc]
            hp = js["hp"]
            nc.tensor.matmul(js["pve"][:], vaug[kc][:, 2 * hp, :],
                             et[:, 0, :], start=(kc == 0), stop=(kc == 7))
            nc.tensor.matmul(js["pvo"][:], vaug[kc][:, 2 * hp + 1, :],
                             et[:, 1, :], start=(kc == 0), stop=(kc == 7))

        def emit_norm(js, split_stg=False):
            qc, hp = js["qc"], js["hp"]
            # Both heads' reciprocals (read straight from the psum sums rows
            # in lane 64) land side by side in one [65,1024] tile; ONE DMA
            # hops that row to a partition-0 tile (partition_broadcast's HW
            # microcode reads partition 0 regardless of the input AP's
            # partition offset) and ONE broadcast serves both heads.  The PV
            # psums are copied to SBUF in parallel, freeing them for the
            # next job's PV.
            rr = recpool.tile([D + 1, 2, 512], F32, tag="rr", name="rr")
            nc.vector.reciprocal(rr[D:D + 1, 0, :], js["pve"][D:D + 1, :])
            nc.vector.reciprocal(rr[D:D + 1, 1, :], js["pvo"][D:D + 1, :])
            rr0 = recpool.tile([1, 2, 512], F32, tag="rr0", name="rr0")
            nc.sync.dma_start(rr0[:], rr[D:D + 1, :, :])
            psbs = []
            for pv in (js["pve"], js["pvo"]):
                psb = pvsb.tile([D + 1, 512], F32, tag="psb", name="psb")
                nc.vector.tensor_copy(psb[:], pv[:])
                psbs.append(psb)
            bcp = bcpool.tile([D, 2, 512], F32, tag="bc", name="bc")
            nc.gpsimd.partition_broadcast(
                bcp[:].rearrange("p a b -> p (a b)"),
                rr0[:].rearrange("p a b -> p (a b)"))
            bcs = [bcp[:, 0, :], bcp[:, 1, :]]
            nc.vector.tensor_mul(
                oT[hp][0:D, qc * 512:(qc + 1) * 512],
                psbs[0][0:D, :], bcs[0])
            # DVE lanes cannot cross partitions: compute head-odd in lanes
            # 0:64, then DMA into partitions 64:128 of the oT tile.  For the
            # final job that move is on the critical path, so split it in two
            # halves so the first DMA overlaps the second multiply.
            stg = stgpool.tile([D, 512], F32R, tag="stg", name="stg")
            nhalf = 2 if split_stg else 1
            w = 512 // nhalf
            for h2 in range(nhalf):
                s = slice(h2 * w, (h2 + 1) * w)
                nc.vector.tensor_mul(stg[:, s], psbs[1][0:D, s], bcs[1][:, s])  # noqa: E501
                nc.sync.dma_start(
                    oT[hp][D:2 * D, qc * 512 + h2 * w:qc * 512 + (h2 + 1) * w],
                    stg[:, s])

        def outproj_mm(tc_i, ic, ops):
            nc.tensor.matmul(
                ops[:],
                oT[ic][:, tc_i * 128:(tc_i + 1) * 128],
                wo[ic][:],
                start=(ic == 0), stop=(ic == 3), skip_group_check=True)

        def outproj_store(tc_i, ops):
            # alternate DVE/ACT for the psum->sbuf copy: at the drain both
            # engines are otherwise idle, so the four stores pipeline 2-wide
            osb = outsb.tile([128, 512], F16, tag="osb", name="osb")
            if tc_i % 2:
                nc.scalar.copy(osb[:], ops[:])
            else:
                nc.vector.tensor_copy(osb[:], ops[:])
            nc.sync.dma_start(out_d[tc_i * 128:(tc_i + 1) * 128, :], osb[:])

        # ---------- schedule ----------
        # Eager startup, emitted in DMA-arrival order so the in-order PE
        # queue never head-of-line blocks: qk(0) interleaved per k-round
        # (each round's matmuls run the moment that round's chunks land),
        # then qk(1) (same inputs, already resident).
        psq = mmps.tile([128, 2, 512], F32, tag="mm", name="psq0")
        psk = mmps.tile([128, 2, 512], F32, tag="mm", name="psk0")
        for k in range(4):
            for n2 in range(2):
                nc.tensor.matmul(
                    psq[:, n2, :], wq[k][:, 0:128],
                    curT[k][:, n2 * 512:(n2 + 1) * 512],
                    start=(k == 0), stop=(k == 3))
                nc.tensor.matmul(
                    psk[:, n2, :], wkv[k][:, 0:128],
                    hidT[k][:, n2 * 512:(n2 + 1) * 512],
                    start=(k == 0), stop=(k == 3))
        nc.vector.tensor_copy(qT[0][:].rearrange("p (a b) -> p a b", a=2), psq[:])
        nc.vector.tensor_copy(kT[0][:].rearrange("p (a b) -> p a b", a=2), psk[:])
        emit_qproj(1)
        emit_kproj(1)
        # vproj fillers occupy job 0 (Wkv-V arrives mid-job-0; every vaug
        # tile exists before job 0's PV runs during job 1); qk(2)/(3) occupy
        # job 1 (m-half1 weight slices arrive just ahead), each ready one
        # job before the head-pair that first consumes it.
        fillers = [lambda m=m, f=f, h=h: f(m, half=h)
                   for m, f in [(0, emit_vproj), (1, emit_vproj),
                                (2, emit_vproj), (3, emit_vproj),
                                (2, emit_qproj), (2, emit_kproj),
                                (3, emit_qproj), (3, emit_kproj)]
                   for h in (0, 1)]
        fillers = fillers[::-1]  # pop() from the front

        # job order: qk[m] is ready exactly when head-pair m first runs;
        # qc=1 norms for hp 0..2 land before the drain.
        jobs = [(0, 0), (1, 0), (0, 1), (0, 2), (0, 3), (1, 1), (1, 2), (1, 3)]

        pending = None
        for ji, (qc, hp) in enumerate(jobs):
            etiles = [epool.tile([128, 2, 512], F32R, tag="exp", name="exp")
                      for _ in range(8)]
            cur = {"qc": qc, "hp": hp, "etiles": etiles, "pve": None,
                   "pvo": None}
            for kt in range(8):
                dp = mmps.tile([128, 2, 512], F32, tag="mm", name="dps")
                nc.tensor.matmul(
                    dp[:, 0, :],
                    kT[hp][0:64, kt * 128:(kt + 1) * 128],
                    qT[hp][0:64, qc * 512:(qc + 1) * 512],
                    start=True, stop=True)
                nc.tensor.matmul(
                    dp[:, 1, :],
                    kT[hp][64:128, kt * 128:(kt + 1) * 128],
                    qT[hp][64:128, qc * 512:(qc + 1) * 512],
                    start=True, stop=True)
                nc.scalar.activation(
                    etiles[kt][:].rearrange("p a b -> p (a b)"),
                    dp[:].rearrange("p a b -> p (a b)"),
                    mybir.ActivationFunctionType.Exp)
                # PV of the previous job overlaps this tile's ACT exp
                if pending is not None:
                    emit_pv(pending, kt)
                if fillers:
                    fillers.pop()()
            if pending is not None:
                emit_norm(pending)
            pending = cur
        # ---------- drain ----------
        for kt in range(8):
            emit_pv(pending, kt)
        # dots/proj psums are done; hand their banks to a 6-buffer outproj
        # pool.  qc=0 tiles (0..3) have no dependence on the final norm and
        # run fully pre-norm; qc=1 tiles (4..7) pre-start ic 0..2 partials.
        mmps_cm.__exit__(None, None, None)
        with tc.tile_pool(name="opbig", bufs=6, space="PSUM") as opbig:
            part = {}
            for t2 in (4, 5):
                part[t2] = opbig.tile([128, 512], F32, tag="op", name="ops")
                for ic in range(3):
                    outproj_mm(t2, ic, part[t2])
            for t2 in (0, 1, 2, 3):
                ops = opbig.tile([128, 512], F32, tag="op", name="ops")
                for ic in range(4):
                    outproj_mm(t2, ic, ops)
                outproj_store(t2, ops)
            for t2 in (6, 7):
                part[t2] = opbig.tile([128, 512], F32, tag="op", name="ops")
                for ic in range(3):
                    outproj_mm(t2, ic, part[t2])
            emit_norm(pending, split_stg=True)
            for t2 in (4, 5, 6, 7):
                outproj_mm(t2, 3, part[t2])
                outproj_store(t2, part[t2])


def make_in_maps(inputs):
    current = np.asarray(inputs["current"], dtype=np.float32)
    hidden = np.asarray(inputs["hidden"], dtype=np.float32)
    Wq = np.ascontiguousarray(np.asarray(inputs["Wq"], dtype=np.float32))
    Wkv = np.ascontiguousarray(np.asarray(inputs["Wkv"], dtype=np.float32))
    Wo = np.ascontiguousarray(np.asarray(inputs["Wo"], dtype=np.float32))

    in_maps = []
    for i in range(NCORES):
        in_maps.append({
            "currentT": np.ascontiguousarray(current[i].T),
            "hiddenT": np.ascontiguousarray(hidden[i].T),
            "Wq": Wq, "Wkv": Wkv, "Wo": Wo,
        })
    return in_maps


def kernel(current, hidden, Wq, Wkv, Wo):
    in_maps = make_in_maps(
        {"current": current, "hidden": hidden, "Wq": Wq, "Wkv": Wkv, "Wo": Wo})
    nc = build_nc()
    res = run_bass_kernel_spmd(nc, in_maps, core_ids=list(range(NCORES)))
    out = np.stack([res.results[i]["out"].astype(np.float32)
                    for i in range(NCORES)], axis=0)
    return out


# revision 56
# speedup vs baseline: 1.3550x; 1.3550x over previous
"""Trainium2 Bass kernel for multi-head attention (B=8, N=1024, DM=512, H=8, D=64).

Sharding: data-parallel over batch — core i handles batch element i, weights
replicated, no collectives.

Per-core pipeline (all matmul operands float32r — tf32-like, 1 cyc/row):
  - host feeds current^T / hidden^T [512, 1024]
  - qT, kT = W^T @ x^T   (weights stationary)      [512 rows, 1024 tokens]
  - v natural [1024, 512] (hiddenT chunks stationary), stored ones-augmented
    as [128, 8, 65] per token-chunk (col 64 = 1.0 -> softmax sums for free)
  - per (q-chunk, head-pair): dotsT [keys, queries], two heads packed
    concurrently in the PE array (K=64 row tiling, banks 0/1 of a 2-bank
    PSUM tile); exp on ScalarE (no max subtraction -- logits are O(+-50),
    fp32 exp cannot overflow) -> SBUF f32r
  - PV: out_h^T[65, 512] = v_aug^T @ expT, row 64 = softmax denominators
  - normalize: gather sums [1,512]->[8,64] (DMA), DVE reciprocal, scatter
    back to a row, GpSimd partition-broadcast [64,512], DVE multiply
  - out-proj: out[t-chunk, :] = sum_ic oT[ic, t-chunk]^T @ Wo[ic, :]
"""
import sys

sys.path.insert(0, "/opt/trn_rl_repo")

import numpy as np

import concourse.bass as bass  # noqa: F401  (import keeps bass registered)
import concourse.mybir as mybir
import concourse.tile as tile
from concourse import bacc
from concourse.bass_utils import run_bass_kernel_spmd

F32 = mybir.dt.float32
F32R = mybir.dt.float32r

B, N, DM = 8, 1024, 512
H, D = 8, 64
NCORES = 8

_nc_cache = {}


def build_nc(loop_n=None):
    key = ("nc", loop_n)
    if key in _nc_cache:
        return _nc_cache[key]
    nc = bacc.Bacc("TRN2", target_bir_lowering=False, debug=False)

    curT_d = nc.dram_tensor("currentT", [DM, N], F32R, kind="ExternalInput").ap()
    hidT_d = nc.dram_tensor("hiddenT", [DM, N], F32R, kind="ExternalInput").ap()
    wq_d = nc.dram_tensor("Wq", [DM, H * D], F32R, kind="ExternalInput").ap()
    wkv_d = nc.dram_tensor("Wkv", [DM, 2 * H * D], F32R, kind="ExternalInput").ap()
    wo_d = nc.dram_tensor("Wo", [H * D, H * D], F32R, kind="ExternalInput").ap()
    ones_d = nc.dram_tensor("ones", [128, H], F32R, kind="ExternalInput").ap()
    out_d = nc.dram_tensor("out", [N, H * D], F32, kind="ExternalOutput").ap()

    with tile.TileContext(nc) as tc:
        if loop_n is None:
            build_body(nc, tc, curT_d, hidT_d, wq_d, wkv_d, wo_d, ones_d, out_d)
        else:
            with tc.For_i(0, loop_n, 1):
                build_body(nc, tc, curT_d, hidT_d, wq_d, wkv_d, wo_d, ones_d,
                           out_d)
    nc.compile()
    _nc_cache[key] = nc
    return nc


CONFIG = {"dots_banks": 2}


def build_body(nc, tc, curT_d, hidT_d, wq_d, wkv_d, wo_d, ones_d, out_d):
    import contextlib

    dots_banks = CONFIG["dots_banks"]
    kpt = dots_banks // 2          # k-chunks per dots tile
    ntiles = 8 // kpt              # dots tiles per pair-job
    dots_bufs = 4 // dots_banks if dots_banks == 4 else 2

    ctx = contextlib.ExitStack()
    with ctx:
        # ---------- pools ----------
        wpool = ctx.enter_context(tc.tile_pool(name="weights", bufs=1))
        actpool = ctx.enter_context(tc.tile_pool(name="acts", bufs=1))
        qkpool = ctx.enter_context(tc.tile_pool(name="qk", bufs=1))
        vpool = ctx.enter_context(tc.tile_pool(name="vaug", bufs=1))
        opool = ctx.enter_context(tc.tile_pool(name="ot", bufs=1))
        epool = ctx.enter_context(tc.tile_pool(name="expT", bufs=24 // dots_banks))
        pvsb = ctx.enter_context(tc.tile_pool(name="pvsb", bufs=3))
        stgpool = ctx.enter_context(tc.tile_pool(name="stg", bufs=2))
        smallp = ctx.enter_context(tc.tile_pool(name="small", bufs=3))
        bcpool = ctx.enter_context(tc.tile_pool(name="bcast", bufs=3))
        outsb = ctx.enter_context(tc.tile_pool(name="outsb", bufs=2))
        dpsum = ctx.enter_context(
            tc.tile_pool(name="dpsum", bufs=dots_bufs, space="PSUM"))
        pvps = ctx.enter_context(tc.tile_pool(name="pvps", bufs=2, space="PSUM"))
        ppsum_cm = tc.tile_pool(name="ppsum", bufs=1, space="PSUM")
        ppsum = ppsum_cm.__enter__()
        state = {"opps": None, "ppsum_open": True}

        # ---------- input loads (spread across DMA queues) ----------
        wq = [wpool.tile([128, 512], F32R, tag=f"wq{k}", name=f"wq{k}") for k in range(4)]
        wkv = [wpool.tile([128, 1024], F32R, tag=f"wkv{k}", name=f"wkv{k}") for k in range(4)]
        wo = [wpool.tile([128, 512], F32R, tag=f"wo{k}", name=f"wo{k}") for k in range(4)]
        curT = [actpool.tile([128, 1024], F32R, tag=f"cur{k}", name=f"cur{k}") for k in range(4)]
        hidT = [actpool.tile([128, 1024], F32R, tag=f"hid{k}", name=f"hid{k}") for k in range(4)]

        hw = [nc.sync, nc.scalar]
        for k in range(4):
            hw[k % 2].dma_start(wq[k][:], wq_d[k * 128:(k + 1) * 128, :])
            hw[(k + 1) % 2].dma_start(curT[k][:], curT_d[k * 128:(k + 1) * 128, :])
        for k in range(4):  # Wk half first: dots need it, Wv only later
            hw[k % 2].dma_start(wkv[k][:, 0:512],
                                wkv_d[k * 128:(k + 1) * 128, 0:512])
            hw[(k + 1) % 2].dma_start(hidT[k][:], hidT_d[k * 128:(k + 1) * 128, :])
        for k in range(4):
            hw[k % 2].dma_start(wkv[k][:, 512:1024],
                                wkv_d[k * 128:(k + 1) * 128, 512:1024])
        for k in range(4):
            hw[k % 2].dma_start(wo[k][:], wo_d[k * 128:(k + 1) * 128, :])

        # ---------- persistent tensors ----------
        qT = [qkpool.tile([128, 1024], F32R, tag=f"qT{m}", name=f"qT{m}") for m in range(4)]
        kT = [qkpool.tile([128, 1024], F32R, tag=f"kT{m}", name=f"kT{m}") for m in range(4)]
        vaug = [vpool.tile([128, H, D + 1], F32R, tag=f"va{t}", name=f"va{t}") for t in range(8)]
        oT = [opool.tile([128, 1024], F32R, tag=f"oT{i}", name=f"oT{i}") for i in range(4)]

        # ---------- projection units (emitted eagerly or as fillers) ----------
        def emit_qproj(m):
            ps = ppsum.tile([128, 2, 512], F32, tag="proj", name="psq")
            for k in range(4):
                for n2 in range(2):
                    nc.tensor.matmul(
                        ps[:, n2, :],
                        wq[k][:, m * 128:(m + 1) * 128],
                        curT[k][:, n2 * 512:(n2 + 1) * 512],
                        start=(k == 0), stop=(k == 3))
            nc.vector.tensor_copy(qT[m][:].rearrange("p (a b) -> p a b", a=2), ps[:])

        def emit_kproj(m):
            ps = ppsum.tile([128, 2, 512], F32, tag="proj", name="psk")
            for k in range(4):
                for n2 in range(2):
                    nc.tensor.matmul(
                        ps[:, n2, :],
                        wkv[k][:, m * 128:(m + 1) * 128],
                        hidT[k][:, n2 * 512:(n2 + 1) * 512],
                        start=(k == 0), stop=(k == 3))
            nc.vector.tensor_copy(kT[m][:].rearrange("p (a b) -> p a b", a=2), ps[:])

        def emit_vproj(tp):
            ps = ppsum.tile([128, 2, 512], F32, tag="proj", name="psv")
            for t2 in range(2):
                tc_i = tp * 2 + t2
                for k in range(4):
                    nc.tensor.matmul(
                        ps[:, t2, :],
                        hidT[k][:, tc_i * 128:(tc_i + 1) * 128],
                        wkv[k][:, 512:1024],
                        start=(k == 0), stop=(k == 3))
            for t2 in range(2):
                tc_i = tp * 2 + t2
                nc.vector.tensor_copy(
                    vaug[tc_i][:, :, 0:D],
                    ps[:, t2, :].rearrange("p (h d) -> p h d", h=H))
                nc.sync.dma_start(vaug[tc_i][:, :, D:D + 1], ones_d[:])

        # ---------- attention helpers ----------
        def emit_pv(js, kc):
            if kc == 0:
                js["pve"] = pvps.tile([D + 1, 512], F32, tag="pv", name="pve")
                js["pvo"] = pvps.tile([D + 1, 512], F32, tag="pv", name="pvo")
            et = js["etiles"][kc // kpt]
            j = kc % kpt
            hp = js["hp"]
            nc.tensor.matmul(js["pve"][:], vaug[kc][:, 2 * hp, :],
                             et[:, 2 * j, :], start=(kc == 0), stop=(kc == 7))
            nc.tensor.matmul(js["pvo"][:], vaug[kc][:, 2 * hp + 1, :],
                             et[:, 2 * j + 1, :], start=(kc == 0), stop=(kc == 7))

        def emit_pv_tile(js, kt):
            for j in range(kpt):
                emit_pv(js, kpt * kt + j)

        def emit_norm(js):
            qc, hp = js["qc"], js["hp"]
            psb_e = pvsb.tile([D + 1, 512], F32, tag="pvsb", name="psbe")
            nc.vector.tensor_copy(psb_e[:], js["pve"][:])
            psb_o = pvsb.tile([D + 1, 512], F32, tag="pvsb", name="psbo")
            nc.vector.tensor_copy(psb_o[:], js["pvo"][:])
            gath = smallp.tile([16, 64], F32, tag="gath", name="gath")
            nc.sync.dma_start(gath[0:8, :], psb_e[D:D + 1, :])
            nc.sync.dma_start(gath[8:16, :], psb_o[D:D + 1, :])
            rec = smallp.tile([16, 64], F32, tag="rec", name="rec")
            nc.vector.reciprocal(rec[:], gath[:])
            for h2, psb in ((0, psb_e), (1, psb_o)):
                rrow = smallp.tile([1, 512], F32, tag="rrow", name="rrow")
                nc.sync.dma_start(rrow[:], rec[8 * h2:8 * h2 + 8, :])
                bc = bcpool.tile([64, 512], F32, tag="bc", name="bc")
                nc.gpsimd.partition_broadcast(bc[:], rrow[:])
                if h2 == 0:
                    nc.vector.tensor_mul(
                        oT[hp][0:64, qc * 512:(qc + 1) * 512],
                        psb[0:D, :], bc[:])
                else:
                    # DVE lanes cannot cross partitions: compute in 0:64,
                    # then DMA into partitions 64:128 of the oT tile.
                    stg = stgpool.tile([64, 512], F32R, tag="stg", name="stg")
                    nc.vector.tensor_mul(stg[:], psb[0:D, :], bc[:])
                    nc.sync.dma_start(
                        oT[hp][64:128, qc * 512:(qc + 1) * 512], stg[:])

        def get_opps():
            if state["opps"] is None:
                state["opps"] = ctx.enter_context(
                    tc.tile_pool(name="opps", bufs=2, space="PSUM"))
            return state["opps"]

        def outproj_start(tc_i, n_ic):
            ops = get_opps().tile([128, 512], F32, tag="op", name="ops")
            for ic in range(n_ic):
                nc.tensor.matmul(
                    ops[:],
                    oT[ic][:, tc_i * 128:(tc_i + 1) * 128],
                    wo[ic][:],
                    start=(ic == 0), stop=False, skip_group_check=True)
            return ops

        def outproj_finish(tc_i, ops, n_ic):
            for ic in range(n_ic, 4):
                nc.tensor.matmul(
                    ops[:],
                    oT[ic][:, tc_i * 128:(tc_i + 1) * 128],
                    wo[ic][:],
                    start=(ic == 0), stop=(ic == 3), skip_group_check=True)
            osb = outsb.tile([128, 512], F32, tag="osb", name="osb")
            nc.vector.tensor_copy(osb[:], ops[:])
            nc.sync.dma_start(out_d[tc_i * 128:(tc_i + 1) * 128, :], osb[:])

        def emit_outproj(qc):
            for t2 in range(4):
                tc_i = qc * 4 + t2
                ops = outproj_start(tc_i, 0)
                outproj_finish(tc_i, ops, 0)

        # ---------- schedule ----------
        # eager: q/k projections for head-pair 0; the rest become fillers
        emit_qproj(0)
        emit_kproj(0)
        fillers = [lambda tp=tp: emit_vproj(tp) for tp in range(4)]
        for m in range(1, 4):
            fillers.append(lambda m=m: emit_qproj(m))
            fillers.append(lambda m=m: emit_kproj(m))
        fillers = fillers[::-1]  # pop() from the front

        # job order: qk[m] is ready exactly when head-pair m first runs
        jobs = [(0, 0), (1, 0), (0, 1), (0, 2), (0, 3), (1, 1), (1, 2), (1, 3)]

        pending = None
        outproj_pending = None
        for ji, (qc, hp) in enumerate(jobs):
            etiles = [epool.tile([128, dots_banks, 512], F32R, tag="exp",
                                 name="exp") for _ in range(ntiles)]
            cur = {"qc": qc, "hp": hp, "etiles": etiles, "pve": None,
                   "pvo": None}
            if not fillers and state["ppsum_open"]:
                state["ppsum_open"] = False
                ppsum_cm.__exit__(None, None, None)
            for kt in range(ntiles):
                dp = dpsum.tile([128, dots_banks, 512], F32, tag="dps",
                                name="dps")
                for j in range(kpt):
                    kc = kpt * kt + j
                    # two heads concurrently: row groups 0:64 and 64:128
                    nc.tensor.matmul(
                        dp[:, 2 * j, :],
                        kT[hp][0:64, kc * 128:(kc + 1) * 128],
                        qT[hp][0:64, qc * 512:(qc + 1) * 512],
                        start=True, stop=True)
                    nc.tensor.matmul(
                        dp[:, 2 * j + 1, :],
                        kT[hp][64:128, kc * 128:(kc + 1) * 128],
                        qT[hp][64:128, qc * 512:(qc + 1) * 512],
                        start=True, stop=True)
                nc.scalar.activation(
                    etiles[kt][:].rearrange("p a b -> p (a b)"),
                    dp[:].rearrange("p a b -> p (a b)"),
                    mybir.ActivationFunctionType.Exp)
                # PV of the previous job overlaps this tile's ACT exp
                if pending is not None:
                    emit_pv_tile(pending, kt)
                if fillers:
                    fillers.pop()()
            if pending is not None:
                emit_norm(pending)
                if pending["qc"] == 0 and pending["hp"] == 3:
                    outproj_pending = 0
            pending = cur
            if outproj_pending is not None:
                emit_outproj(outproj_pending)
                outproj_pending = None
        # drain last job; overlap outproj(1) partials (ic 0-2 need only
        # oT[0..2], which were normalized in earlier jobs)
        for kt in range(ntiles):
            emit_pv_tile(pending, kt)
        part = [outproj_start(4 + t2, 3) for t2 in range(2)]
        emit_norm(pending)
        for t2 in range(2):
            outproj_finish(4 + t2, part[t2], 3)
        for t2 in range(2, 4):
            ops = outproj_start(4 + t2, 0)
            outproj_finish(4 + t2, ops, 0)


def make_in_maps(inputs):
    current = np.asarray(inputs["current"], dtype=np.float32)
    hidden = np.asarray(inputs["hidden"], dtype=np.float32)
    Wq = np.ascontiguousarray(np.asarray(inputs["Wq"], dtype=np.float32))
    Wkv = np.ascontiguousarray(np.asarray(inputs["Wkv"], dtype=np.float32))
    Wo = np.ascontiguousarray(np.asarray(inputs["Wo"], dtype=np.float32))
    ones = np.ones((128, H), dtype=np.float32)

    in_maps = []
    for i in range(NCORES):
        in_maps.append({
            "currentT": np.ascontiguousarray(current[i].T),
            "hiddenT": np.ascontiguousarray(hidden[i].T),
            "Wq": Wq, "Wkv": Wkv, "Wo": Wo, "ones": ones,
        })
    return in_maps


def kernel(current, hidden, Wq, Wkv, Wo):
    in_maps = make_in_maps(
        {"current": current, "hidden": hidden, "Wq": Wq, "Wkv": Wkv, "Wo": Wo})
    nc = build_nc()
    res = run_bass_kernel_spmd(nc, in_maps, core_ids=list(range(NCORES)))
    out = np.stack([res.results[i]["out"] for i in range(NCORES)], axis=0)
    return out

